# revision 36
# baseline (speedup 1.0000x reference)
"""Single-head causal attention (B=8, T=2048, C=1024, H=128) on 8 TRN2 NeuronCores.

Sharding: data-parallel over batch — core b computes batch element b entirely
(no collectives). Host pre-transposes x[b] to xT=[C,T]; the device returns
out^T=[H,T] which the host transposes back.

Schedule: all-f32r PE stream, software-pipelined across chunks. Phase j
emits chunk j's diagonal attention steps, chunk j+1's off-diagonal steps,
and chunk j+1's projection matmuls, woven so the in-order PE queue always
has ready work while ACT exp latency drains. AV and denominator (ones)
matmuls trail their score step by 3 (sc_ps bufs=3), accumulating into
per-chunk ps_o / ps_d banks with causal width-trimming. bk is dropped
(softmax shift invariance); bias adds and PSUM->SBUF copies run on DVE so
ACT is a pure exp stream. Warmup matmuls are woven through the DMA-paced
prologue to hold the PE clock up.
"""

import os
import numpy as np

T, C, H = 2048, 1024, 128
B = 8
P = 128
CT = C // P          # 8 contraction tiles
NCH = 4              # t-chunks
CHW = T // NCH       # 512 chunk width
SPC = CHW // P       # 4 s-tiles per chunk
N_CORES = 8
WARMUP = 12

LAST_EXEC_TIME_NS = None

_BUILT = None


def _build():
    global _BUILT
    if _BUILT is not None:
        return _BUILT

    import concourse.bass as bass  # noqa: F401
    import concourse.mybir as mybir
    from concourse import bacc
    from concourse.tile import TileContext

    F32 = mybir.dt.float32
    F16 = mybir.dt.float16
    F32R = mybir.dt.float32r
    BF16 = mybir.dt.bfloat16
    Identity = mybir.ActivationFunctionType.Identity
    Exp = mybir.ActivationFunctionType.Exp
    Mult = mybir.AluOpType.mult
    Add = mybir.AluOpType.add

    nc = bacc.Bacc()

    xT_ext = nc.declare_dram_parameter("xT", [C, T], F16, isOutput=False)
    w_ext = {
        n: nc.declare_dram_parameter(n, [C, H], F16, isOutput=False)
        for n in ("Wq", "Wk", "Wv")
    }
    b_ext = {
        n: nc.declare_dram_parameter(n, [H, 1], F32, isOutput=False)
        for n in ("bq", "bv")
    }
    tril_ext = nc.declare_dram_parameter("tril", [P, P], BF16, isOutput=False)
    ones_ext = nc.declare_dram_parameter("ones", [P, P], F32R, isOutput=False)
    ident_ext = nc.declare_dram_parameter("ident", [P, P], F32R, isOutput=False)
    out_ext = nc.declare_dram_parameter("out", [H, T], F32, isOutput=True)

    xT_r = xT_ext.rearrange("(ct p) t -> p ct t", p=P)
    w_r = {n: w_ext[n].rearrange("(ct p) h -> p ct h", p=P) for n in w_ext}

    with TileContext(nc) as tc:
        with (
            tc.tile_pool(name="const", bufs=1) as const,
            tc.tile_pool(name="kt", bufs=NCH) as kt_pool,
            tc.tile_pool(name="vnat", bufs=16) as v_pool,
            tc.tile_pool(name="xch", bufs=2) as x_pool,
            tc.tile_pool(name="qv", bufs=2) as qv_pool,
            tc.tile_pool(name="ex", bufs=8) as e_pool,
            tc.tile_pool(name="outp", bufs=2) as out_pool,
            tc.tile_pool(name="ps_proj", bufs=1, space="PSUM") as proj_ps,
            tc.tile_pool(name="ps_sc", bufs=3, space="PSUM") as sc_ps,
            tc.tile_pool(name="ps_o", bufs=2, space="PSUM") as o_ps,
            tc.tile_pool(name="ps_d", bufs=1, space="PSUM") as d_ps,
            tc.tile_pool(name="ps_tr", bufs=1, space="PSUM") as tr_ps,
        ):
            # ---- constants; ones/ident first (warmup), then Wq + x0
            w_sb = {}
            for n in ("Wq", "Wk", "Wv"):
                w_sb[n] = [
                    const.tile([P, H], F16, tag=f"w_{n}_{c}", name=f"w_{n}_{c}")
                    for c in range(CT)
                ]
            b_sb = {
                n: const.tile([H, 1], F32, tag=f"b_{n}", name=f"b_{n}")
                for n in ("bq", "bv")
            }
            ones_r = const.tile([P, P], F32R, tag="ones_r")
            nc.sync.dma_start(ones_r[:], ones_ext[:])
            ident = const.tile([P, P], F32R, tag="ident")
            nc.sync.dma_start(ident[:], ident_ext[:])
            x_tiles = [None] * NCH

            def emit_x_dma(j):
                tiles = []
                tsl = slice(CHW * j, CHW * (j + 1))
                for c in range(CT):
                    xt = x_pool.tile([P, CHW], F16, tag=f"xc{c}", name=f"x{j}_{c}")
                    nc.sync.dma_start(xt[:], xT_r[:, c, tsl])
                    tiles.append(xt)
                x_tiles[j] = tiles

            for c in range(CT):
                nc.sync.dma_start(w_sb["Wq"][c][:], w_r["Wq"][:, c, :])
            emit_x_dma(0)
            nc.sync.dma_start(b_sb["bq"][:], b_ext["bq"][:])
            nc.sync.dma_start(b_sb["bv"][:], b_ext["bv"][:])
            tril = const.tile([P, P], BF16, tag="tril")
            nc.sync.dma_start(tril[:], tril_ext[:])
            for c in range(CT):
                nc.sync.dma_start(w_sb["Wk"][c][:], w_r["Wk"][:, c, :])
            for c in range(CT):
                nc.sync.dma_start(w_sb["Wv"][c][:], w_r["Wv"][:, c, :])
            emit_x_dma(1)

            # PE warmup over the DMA prologue; bf16 memset tiles so the PE
            # can start before any DMA lands.
            ones_bf = const.tile([P, P], BF16, tag="ones_bf")
            nc.vector.memset(ones_bf[:], 1.0)
            warm_src = const.tile([P, CHW], BF16, tag="warm_src")
            nc.vector.memset(warm_src[:], 0.0)
            ps_warm = sc_ps.tile([P, CHW], F32, tag="sc", name="ps_warm")

            def warm(n=1, width=CHW):
                for _ in range(n):
                    nc.tensor.matmul(
                        ps_warm[:, :width], ones_bf[:], warm_src[:, :width],
                        start=True, stop=True,
                    )

            warm(WARMUP)

            kt_ch = [None] * NCH
            v_tiles = [None] * (NCH * SPC)
            q_chs = [None] * NCH
            state = {}

            def proj_units(j):
                """Unit thunks: q chain(+tail), k chain(+tail), v chain(+tail),
                then the 4 v transposes."""
                units = []
                ps_tiles = {}
                xt = x_tiles[j]

                def mk_mm(kind, wname, c):
                    def f():
                        if c == 0:
                            ps_tiles[kind] = proj_ps.tile(
                                [P, CHW], F32, tag="proj", name=f"ps_{kind}{j}"
                            )
                        nc.tensor.matmul(
                            ps_tiles[kind][:],
                            w_sb[wname][c][:],
                            xt[c][:],
                            start=(c == 0),
                            stop=(c == CT - 1),
                        )
                    return f

                def q_tail():
                    q = qv_pool.tile([P, CHW], F32R, tag="qch", name=f"q{j}")
                    nc.vector.tensor_scalar_add(q[:], ps_tiles["q"][:], b_sb["bq"][:])
                    q_chs[j] = q

                def k_tail():
                    kt = kt_pool.tile([P, CHW], F32R, tag="ktch", name=f"kt{j}")
                    nc.vector.tensor_copy(kt[:], ps_tiles["k"][:])
                    kt_ch[j] = kt

                def v_tail():
                    vch = qv_pool.tile([P, CHW], F32R, tag="vch", name=f"v{j}")
                    nc.vector.tensor_scalar_add(
                        vch[:], ps_tiles["v"][:], b_sb["bv"][:]
                    )
                    ps_tiles["vch"] = vch

                def mk_vt(st):
                    def f():
                        ps_t = tr_ps.tile([P, P], F32R, tag="tr")
                        nc.tensor.transpose(
                            ps_t[:],
                            ps_tiles["vch"][:, P * st : P * (st + 1)],
                            ident[:],
                        )
                        vt = v_pool.tile(
                            [P, P], F32R, tag="vnat", name=f"vnat_{SPC*j+st}"
                        )
                        nc.vector.tensor_copy(vt[:], ps_t[:])
                        v_tiles[SPC * j + st] = vt
                    return f

                for kind, wname, tail in (
                    ("q", "Wq", q_tail),
                    ("k", "Wk", k_tail),
                    ("v", "Wv", v_tail),
                ):
                    for c in range(CT):
                        units.append(mk_mm(kind, wname, c))
                    units.append(tail)
                for st in range(SPC):
                    units.append(mk_vt(st))
                return units

            def make_state(j):
                n_s = SPC * (j + 1)
                n_off = SPC * j
                s = {"n_s": n_s, "n_off": n_off, "pending": []}
                s["ps_o"] = o_ps.tile([P, CHW], F32, tag="o", name=f"o{j}")
                s["ps_d"] = d_ps.tile([P, CHW], F32, tag="d", name=f"d{j}")
                state[j] = s

            def emit_step(j, i):
                s = state[j]
                n_s, n_off = s["n_s"], s["n_off"]
                diag = i >= n_off
                st = i - n_off
                w0 = P * st if diag else 0
                w0sc = w0

                ps_sc = sc_ps.tile([P, CHW], F32, tag="sc", name=f"sc{j}_{i}")
                nc.tensor.matmul(
                    ps_sc[:, w0sc:],
                    kt_ch[i // SPC][:, P * (i % SPC) : P * (i % SPC + 1)],
                    q_chs[j][:, w0sc:],
                    start=True,
                    stop=True,
                )
                eb = e_pool.tile([P, CHW], F32R, tag="e", name=f"e{j}_{i}")
                nc.scalar.activation(eb[:, w0:], ps_sc[:, w0:], Exp)
                if diag:
                    nc.vector.tensor_tensor(
                        eb[:, w0 : w0 + P], eb[:, w0 : w0 + P], tril[:], Mult
                    )

                vt = v_tiles[i]

                def pend(eb=eb, vt=vt, w0=w0, i=i):
                    nc.tensor.matmul(
                        s["ps_o"][:, w0:],
                        vt[:],
                        eb[:, w0:],
                        start=(i == 0),
                        stop=(i == n_s - 1),
                    )
                    nc.tensor.matmul(
                        s["ps_d"][:, w0:], ones_r[:], eb[:, w0:],
                        start=(i == 0),
                        stop=(i == n_s - 1),
                    )
                s["pending"].append(pend)
                if len(s["pending"]) > 3:
                    s["pending"].pop(0)()

            def drain(j, fills=None):
                pend = state[j]["pending"]
                state[j]["pending"] = []
                for n, f in enumerate(pend):
                    f()
                    if fills and n == 0:
                        for g in fills:
                            g()

            def emit_end(j, nsplit):
                tsl0 = CHW * j
                s = state[j]
                recip = out_pool.tile([P, CHW], F32, tag="recip", name=f"rc{j}")
                o_sb = out_pool.tile([P, CHW], F32, tag="osb", name=f"ob{j}")
                w = CHW // nsplit
                for sp in range(nsplit):
                    sl = slice(w * sp, w * (sp + 1))
                    nc.vector.reciprocal_approx_fast(
                        out=recip[:, sl], in_=s["ps_d"][:, sl]
                    )
                    nc.vector.tensor_tensor(
                        o_sb[:, sl], s["ps_o"][:, sl], recip[:, sl], Mult
                    )
                    nc.sync.dma_start(
                        out_ext[:, tsl0 + w * sp : tsl0 + w * (sp + 1)], o_sb[:, sl]
                    )

            # ---- prologue: chunk 0 projections; keep the PE clock up with
            # filler matmuls while the x0 c-tiles land
            make_state(0)
            p0 = proj_units(0)
            for n, u in enumerate(p0):
                if n <= CT:
                    warm(2)
                u()

            # ---- phases: chunk j diagonal + chunk j+1 off-diagonal + proj j+1
            for j in range(NCH):
                has_next = j + 1 < NCH
                if j + 2 < NCH:
                    emit_x_dma(j + 2)
                D = list(range(SPC * j, SPC * (j + 1)))
                if has_next:
                    make_state(j + 1)
                    F = proj_units(j + 1)
                    O = list(range(SPC * (j + 1)))
                else:
                    F, O = [], []

                # part 1: chunk j diagonal steps woven with the start of the
                # next chunk's q projection chain
                q_chain, rest = F[: CT + 1], F[CT + 1 :]
                qi = 0
                for d_idx, i in enumerate(D):
                    emit_step(j, i)
                    if j == 0:
                        continue  # x1 still landing; don't block the PE queue
                    take = (len(q_chain) * (d_idx + 1)) // max(1, len(D)) - qi
                    for _ in range(take):
                        q_chain[qi]()
                        qi += 1
                while qi < len(q_chain):
                    q_chain[qi]()
                    qi += 1
                drain(j, fills=rest[:3])
                rest = rest[3:]
                emit_end(j, 2 if has_next else 4)

                # part 2: chunk j+1 off-diagonal steps woven with its k/v
                # projection chains and v transposes
                # front-load: consume all rest units by ~70% through O
                ri = 0
                n_o = max(1, (len(O) * 7) // 10)
                for o_idx, i in enumerate(O):
                    emit_step(j + 1, i)
                    take = (len(rest) * min(o_idx + 1, n_o)) // n_o - ri
                    for _ in range(take):
                        rest[ri]()
                        ri += 1
                while ri < len(rest):
                    rest[ri]()
                    ri += 1

    nc.compile()
    _BUILT = nc
    return nc


def _host_inputs(x, Wq, bq, Wk, bk, Wv, bv):
    import ml_dtypes

    tril = (np.arange(P)[:, None] <= np.arange(P)[None, :]).astype(
        ml_dtypes.bfloat16
    )
    shared = {
        "Wq": np.ascontiguousarray(Wq, dtype=np.float16),
        "Wk": np.ascontiguousarray(Wk, dtype=np.float16),
        "Wv": np.ascontiguousarray(Wv, dtype=np.float16),
        "bq": np.ascontiguousarray(bq, dtype=np.float32).reshape(H, 1),
        "bv": np.ascontiguousarray(bv, dtype=np.float32).reshape(H, 1),
        "tril": tril,
        "ones": np.ones((P, P), dtype=np.float32),
        "ident": np.eye(P, dtype=np.float32),
    }
    in_maps = []
    for b in range(B):
        m = dict(shared)
        m["xT"] = np.ascontiguousarray(np.asarray(x[b]).T.astype(np.float16))
        in_maps.append(m)
    return in_maps


def kernel(x, Wq, bq, Wk, bk, Wv, bv):
    global LAST_EXEC_TIME_NS
    from concourse.bass_utils import run_bass_kernel_spmd

    nc = _build()
    in_maps = _host_inputs(x, Wq, bq, Wk, bk, Wv, bv)
    trace = os.environ.get("BASS_ATTN_TRACE", "0") == "1"
    res = run_bass_kernel_spmd(nc, in_maps, core_ids=list(range(N_CORES)), trace=trace)
    LAST_EXEC_TIME_NS = res.exec_time_ns
    out = np.stack([res.results[b]["out"].T for b in range(B)], axis=0)
    return np.ascontiguousarray(out, dtype=np.float32)


# revision 37
# speedup vs baseline: 1.0067x; 1.0067x over previous
"""Single-head causal attention (B=8, T=2048, C=1024, H=128) on 8 TRN2 NeuronCores.

Sharding: data-parallel over batch — core b computes batch element b entirely
(no collectives). Host pre-transposes x[b] to xT=[C,T]; the device returns
out^T=[H,T] which the host transposes back.

Schedule: all-f32r PE stream, software-pipelined across chunks. Phase j
emits chunk j's diagonal attention steps, chunk j+1's off-diagonal steps,
and chunk j+1's projection matmuls, woven so the in-order PE queue always
has ready work while ACT exp latency drains. AV and denominator (ones)
matmuls trail their score step by 3 (sc_ps bufs=3), accumulating into
per-chunk ps_o / ps_d banks with causal width-trimming. bk is dropped
(softmax shift invariance); bias adds and PSUM->SBUF copies run on DVE so
ACT is a pure exp stream. Warmup matmuls are woven through the DMA-paced
prologue to hold the PE clock up.
"""

import os
import numpy as np

T, C, H = 2048, 1024, 128
B = 8
P = 128
CT = C // P          # 8 contraction tiles
NCH = 4              # t-chunks
CHW = T // NCH       # 512 chunk width
SPC = CHW // P       # 4 s-tiles per chunk
N_CORES = 8
WARMUP = 10

LAST_EXEC_TIME_NS = None

_BUILT = None


def _build():
    global _BUILT
    if _BUILT is not None:
        return _BUILT

    import concourse.bass as bass  # noqa: F401
    import concourse.mybir as mybir
    from concourse import bacc
    from concourse.tile import TileContext

    F32 = mybir.dt.float32
    F16 = mybir.dt.float16
    F32R = mybir.dt.float32r
    BF16 = mybir.dt.bfloat16
    Identity = mybir.ActivationFunctionType.Identity
    Exp = mybir.ActivationFunctionType.Exp
    Mult = mybir.AluOpType.mult
    Add = mybir.AluOpType.add

    nc = bacc.Bacc()

    xT_ext = nc.declare_dram_parameter("xT", [C, T], F16, isOutput=False)
    w_ext = {
        n: nc.declare_dram_parameter(n, [C, H], F16, isOutput=False)
        for n in ("Wq", "Wk", "Wv")
    }
    b_ext = {
        n: nc.declare_dram_parameter(n, [H, 1], F32, isOutput=False)
        for n in ("bq", "bv")
    }
    tril_ext = nc.declare_dram_parameter("tril", [P, P], BF16, isOutput=False)
    ones_ext = nc.declare_dram_parameter("ones", [P, P], F32R, isOutput=False)
    ident_ext = nc.declare_dram_parameter("ident", [P, P], F32R, isOutput=False)
    out_ext = nc.declare_dram_parameter("out", [H, T], F32, isOutput=True)

    xT_r = xT_ext.rearrange("(ct p) t -> p ct t", p=P)
    w_r = {n: w_ext[n].rearrange("(ct p) h -> p ct h", p=P) for n in w_ext}

    with TileContext(nc) as tc:
        with (
            tc.tile_pool(name="const", bufs=1) as const,
            tc.tile_pool(name="kt", bufs=NCH) as kt_pool,
            tc.tile_pool(name="vnat", bufs=16) as v_pool,
            tc.tile_pool(name="xch", bufs=2) as x_pool,
            tc.tile_pool(name="qv", bufs=2) as qv_pool,
            tc.tile_pool(name="ex", bufs=8) as e_pool,
            tc.tile_pool(name="outp", bufs=2) as out_pool,
            tc.tile_pool(name="ps_proj", bufs=1, space="PSUM") as proj_ps,
            tc.tile_pool(name="ps_sc", bufs=3, space="PSUM") as sc_ps,
            tc.tile_pool(name="ps_o", bufs=2, space="PSUM") as o_ps,
            tc.tile_pool(name="ps_d", bufs=1, space="PSUM") as d_ps,
            tc.tile_pool(name="ps_tr", bufs=1, space="PSUM") as tr_ps,
        ):
            # ---- constants; ones/ident first (warmup), then Wq + x0
            w_sb = {}
            for n in ("Wq", "Wk", "Wv"):
                w_sb[n] = [
                    const.tile([P, H], F16, tag=f"w_{n}_{c}", name=f"w_{n}_{c}")
                    for c in range(CT)
                ]
            b_sb = {
                n: const.tile([H, 1], F32, tag=f"b_{n}", name=f"b_{n}")
                for n in ("bq", "bv")
            }
            ones_r = const.tile([P, P], F32R, tag="ones_r")
            nc.sync.dma_start(ones_r[:], ones_ext[:])
            ident = const.tile([P, P], F32R, tag="ident")
            nc.sync.dma_start(ident[:], ident_ext[:])
            x_tiles = [None] * NCH

            def emit_x_dma(j):
                tiles = []
                tsl = slice(CHW * j, CHW * (j + 1))
                for c in range(CT):
                    xt = x_pool.tile([P, CHW], F16, tag=f"xc{c}", name=f"x{j}_{c}")
                    nc.sync.dma_start(xt[:], xT_r[:, c, tsl])
                    tiles.append(xt)
                x_tiles[j] = tiles

            for c in range(CT):
                nc.sync.dma_start(w_sb["Wq"][c][:], w_r["Wq"][:, c, :])
            emit_x_dma(0)
            nc.sync.dma_start(b_sb["bq"][:], b_ext["bq"][:])
            nc.sync.dma_start(b_sb["bv"][:], b_ext["bv"][:])
            tril = const.tile([P, P], BF16, tag="tril")
            nc.sync.dma_start(tril[:], tril_ext[:])
            for c in range(CT):
                nc.sync.dma_start(w_sb["Wk"][c][:], w_r["Wk"][:, c, :])
            for c in range(CT):
                nc.sync.dma_start(w_sb["Wv"][c][:], w_r["Wv"][:, c, :])
            emit_x_dma(1)

            # PE warmup over the DMA prologue; bf16 memset tiles so the PE
            # can start before any DMA lands.
            ones_bf = const.tile([P, P], BF16, tag="ones_bf")
            nc.vector.memset(ones_bf[:], 1.0)
            warm_src = const.tile([P, CHW], BF16, tag="warm_src")
            nc.vector.memset(warm_src[:], 0.0)
            ps_warm = sc_ps.tile([P, CHW], F32, tag="sc", name="ps_warm")

            def warm(n=1, width=CHW):
                for _ in range(n):
                    nc.tensor.matmul(
                        ps_warm[:, :width], ones_bf[:], warm_src[:, :width],
                        start=True, stop=True,
                    )

            warm(WARMUP)

            kt_ch = [None] * NCH
            v_tiles = [None] * (NCH * SPC)
            q_chs = [None] * NCH
            state = {}

            def proj_units(j):
                """Unit thunks: q chain(+tail), k chain(+tail), v chain(+tail),
                then the 4 v transposes."""
                units = []
                ps_tiles = {}
                xt = x_tiles[j]

                def mk_mm(kind, wname, c):
                    def f():
                        if c == 0:
                            ps_tiles[kind] = proj_ps.tile(
                                [P, CHW], F32, tag="proj", name=f"ps_{kind}{j}"
                            )
                        nc.tensor.matmul(
                            ps_tiles[kind][:],
                            w_sb[wname][c][:],
                            xt[c][:],
                            start=(c == 0),
                            stop=(c == CT - 1),
                        )
                    return f

                def q_tail():
                    q = qv_pool.tile([P, CHW], F32R, tag="qch", name=f"q{j}")
                    nc.vector.tensor_scalar_add(q[:], ps_tiles["q"][:], b_sb["bq"][:])
                    q_chs[j] = q

                def k_tail():
                    kt = kt_pool.tile([P, CHW], F32R, tag="ktch", name=f"kt{j}")
                    nc.vector.tensor_copy(kt[:], ps_tiles["k"][:])
                    kt_ch[j] = kt

                def v_tail():
                    vch = qv_pool.tile([P, CHW], F32R, tag="vch", name=f"v{j}")
                    nc.vector.tensor_scalar_add(
                        vch[:], ps_tiles["v"][:], b_sb["bv"][:]
                    )
                    ps_tiles["vch"] = vch

                def mk_vt(st):
                    def f():
                        ps_t = tr_ps.tile([P, P], F32R, tag="tr")
                        nc.tensor.transpose(
                            ps_t[:],
                            ps_tiles["vch"][:, P * st : P * (st + 1)],
                            ident[:],
                        )
                        vt = v_pool.tile(
                            [P, P], F32R, tag="vnat", name=f"vnat_{SPC*j+st}"
                        )
                        nc.vector.tensor_copy(vt[:], ps_t[:])
                        v_tiles[SPC * j + st] = vt
                    return f

                for kind, wname, tail in (
                    ("q", "Wq", q_tail),
                    ("k", "Wk", k_tail),
                    ("v", "Wv", v_tail),
                ):
                    for c in range(CT):
                        units.append(mk_mm(kind, wname, c))
                    units.append(tail)
                for st in range(SPC):
                    units.append(mk_vt(st))
                return units

            def make_state(j):
                n_s = SPC * (j + 1)
                n_off = SPC * j
                s = {"n_s": n_s, "n_off": n_off, "pending": []}
                s["ps_o"] = o_ps.tile([P, CHW], F32, tag="o", name=f"o{j}")
                s["ps_d"] = d_ps.tile([P, CHW], F32, tag="d", name=f"d{j}")
                state[j] = s

            def emit_step(j, i):
                s = state[j]
                n_s, n_off = s["n_s"], s["n_off"]
                diag = i >= n_off
                st = i - n_off
                w0 = P * st if diag else 0
                w0sc = w0

                ps_sc = sc_ps.tile([P, CHW], F32, tag="sc", name=f"sc{j}_{i}")
                nc.tensor.matmul(
                    ps_sc[:, w0sc:],
                    kt_ch[i // SPC][:, P * (i % SPC) : P * (i % SPC + 1)],
                    q_chs[j][:, w0sc:],
                    start=True,
                    stop=True,
                )
                eb = e_pool.tile([P, CHW], F32R, tag="e", name=f"e{j}_{i}")
                nc.scalar.activation(eb[:, w0:], ps_sc[:, w0:], Exp)
                if diag:
                    nc.vector.tensor_tensor(
                        eb[:, w0 : w0 + P], eb[:, w0 : w0 + P], tril[:], Mult
                    )

                vt = v_tiles[i]

                def pend(eb=eb, vt=vt, w0=w0, i=i):
                    nc.tensor.matmul(
                        s["ps_o"][:, w0:],
                        vt[:],
                        eb[:, w0:],
                        start=(i == 0),
                        stop=(i == n_s - 1),
                    )
                    nc.tensor.matmul(
                        s["ps_d"][:, w0:], ones_r[:], eb[:, w0:],
                        start=(i == 0),
                        stop=(i == n_s - 1),
                    )
                s["pending"].append(pend)
                if len(s["pending"]) > 3:
                    s["pending"].pop(0)()

            def drain(j, fills=None):
                pend = state[j]["pending"]
                state[j]["pending"] = []
                for n, f in enumerate(pend):
                    f()
                    if fills and n == 0:
                        for g in fills:
                            g()

            def emit_end(j, nsplit):
                tsl0 = CHW * j
                s = state[j]
                recip = out_pool.tile([P, CHW], F32, tag="recip", name=f"rc{j}")
                o_sb = out_pool.tile([P, CHW], F32, tag="osb", name=f"ob{j}")
                w = CHW // nsplit
                for sp in range(nsplit):
                    sl = slice(w * sp, w * (sp + 1))
                    nc.vector.reciprocal_approx_fast(
                        out=recip[:, sl], in_=s["ps_d"][:, sl]
                    )
                    nc.vector.tensor_tensor(
                        o_sb[:, sl], s["ps_o"][:, sl], recip[:, sl], Mult
                    )
                    nc.sync.dma_start(
                        out_ext[:, tsl0 + w * sp : tsl0 + w * (sp + 1)], o_sb[:, sl]
                    )

            # ---- prologue: chunk 0 projections; keep the PE clock up with
            # filler matmuls while the x0 c-tiles land
            make_state(0)
            p0 = proj_units(0)
            for n, u in enumerate(p0):
                if n <= CT:
                    warm(3)
                u()

            # ---- phases: chunk j diagonal + chunk j+1 off-diagonal + proj j+1
            for j in range(NCH):
                has_next = j + 1 < NCH
                if j + 2 < NCH:
                    emit_x_dma(j + 2)
                D = list(range(SPC * j, SPC * (j + 1)))
                if has_next:
                    make_state(j + 1)
                    F = proj_units(j + 1)
                    O = list(range(SPC * (j + 1)))
                else:
                    F, O = [], []

                # part 1: chunk j diagonal steps woven with the start of the
                # next chunk's q projection chain
                q_chain, rest = F[: CT + 1], F[CT + 1 :]
                qi = 0
                for d_idx, i in enumerate(D):
                    emit_step(j, i)
                    if j == 0:
                        continue  # x1 still landing; don't block the PE queue
                    take = (len(q_chain) * (d_idx + 1)) // max(1, len(D)) - qi
                    for _ in range(take):
                        q_chain[qi]()
                        qi += 1
                while qi < len(q_chain):
                    q_chain[qi]()
                    qi += 1
                drain(j, fills=rest[:2])
                rest = rest[2:]
                emit_end(j, 2 if has_next else 4)

                # part 2: chunk j+1 off-diagonal steps woven with its k/v
                # projection chains and v transposes
                # front-load: consume all rest units by ~70% through O
                ri = 0
                n_o = max(1, (len(O) * 7) // 10)
                for o_idx, i in enumerate(O):
                    emit_step(j + 1, i)
                    if j == 0:
                        warm(1)
                    take = (len(rest) * min(o_idx + 1, n_o)) // n_o - ri
                    for _ in range(take):
                        rest[ri]()
                        ri += 1
                while ri < len(rest):
                    rest[ri]()
                    ri += 1

    nc.compile()
    _BUILT = nc
    return nc


def _host_inputs(x, Wq, bq, Wk, bk, Wv, bv):
    import ml_dtypes

    tril = (np.arange(P)[:, None] <= np.arange(P)[None, :]).astype(
        ml_dtypes.bfloat16
    )
    shared = {
        "Wq": np.ascontiguousarray(Wq, dtype=np.float16),
        "Wk": np.ascontiguousarray(Wk, dtype=np.float16),
        "Wv": np.ascontiguousarray(Wv, dtype=np.float16),
        "bq": np.ascontiguousarray(bq, dtype=np.float32).reshape(H, 1),
        "bv": np.ascontiguousarray(bv, dtype=np.float32).reshape(H, 1),
        "tril": tril,
        "ones": np.ones((P, P), dtype=np.float32),
        "ident": np.eye(P, dtype=np.float32),
    }
    in_maps = []
    for b in range(B):
        m = dict(shared)
        m["xT"] = np.ascontiguousarray(np.asarray(x[b]).T.astype(np.float16))
        in_maps.append(m)
    return in_maps


def kernel(x, Wq, bq, Wk, bk, Wv, bv):
    global LAST_EXEC_TIME_NS
    from concourse.bass_utils import run_bass_kernel_spmd

    nc = _build()
    in_maps = _host_inputs(x, Wq, bq, Wk, bk, Wv, bv)
    trace = os.environ.get("BASS_ATTN_TRACE", "0") == "1"
    res = run_bass_kernel_spmd(nc, in_maps, core_ids=list(range(N_CORES)), trace=trace)
    LAST_EXEC_TIME_NS = res.exec_time_ns
    out = np.stack([res.results[b]["out"].T for b in range(B)], axis=0)
    return np.ascontiguousarray(out, dtype=np.float32)


# revision 38
# speedup vs baseline: 1.0645x; 1.0574x over previous
"""Single-head causal attention (B=8, T=2048, C=1024, H=128) on 8 TRN2 NeuronCores.

Sharding: data-parallel over batch — core b computes batch element b entirely
(no collectives). Host pre-transposes x[b] to xT=[C,T]; the device returns
out^T=[H,T] which the host transposes back.

Schedule: all-f32r PE stream, software-pipelined across chunks. Phase j
emits chunk j's diagonal attention steps, chunk j+1's off-diagonal steps,
and chunk j+1's projection matmuls, woven so the in-order PE queue always
has ready work while ACT exp latency drains. AV and denominator (ones)
matmuls trail their score step by 3 (sc_ps bufs=3), accumulating into
per-chunk ps_o / ps_d banks with causal width-trimming. bk is dropped
(softmax shift invariance); bias adds and PSUM->SBUF copies run on DVE so
ACT is a pure exp stream. Warmup matmuls are woven through the DMA-paced
prologue to hold the PE clock up.
"""

import os
import numpy as np

T, C, H = 2048, 1024, 128
B = 8
P = 128
CT = C // P          # 8 contraction tiles
NCH = 4              # t-chunks
CHW = T // NCH       # 512 chunk width
SPC = CHW // P       # 4 s-tiles per chunk
N_CORES = 8
WARMUP = 10

LAST_EXEC_TIME_NS = None

_BUILT = None


def _build():
    global _BUILT
    if _BUILT is not None:
        return _BUILT

    import concourse.bass as bass  # noqa: F401
    import concourse.mybir as mybir
    from concourse import bacc
    from concourse.tile import TileContext

    F32 = mybir.dt.float32
    F16 = mybir.dt.float16
    F32R = mybir.dt.float32r
    BF16 = mybir.dt.bfloat16
    Identity = mybir.ActivationFunctionType.Identity
    Exp = mybir.ActivationFunctionType.Exp
    Mult = mybir.AluOpType.mult
    Add = mybir.AluOpType.add

    nc = bacc.Bacc()

    xT_ext = nc.declare_dram_parameter("xT", [C, T], F16, isOutput=False)
    w_ext = {
        n: nc.declare_dram_parameter(n, [C, H], F16, isOutput=False)
        for n in ("Wq", "Wk", "Wv")
    }
    b_ext = {
        n: nc.declare_dram_parameter(n, [H, 1], F32, isOutput=False)
        for n in ("bq", "bv")
    }
    tril_ext = nc.declare_dram_parameter("tril", [P, P], BF16, isOutput=False)
    ones_ext = nc.declare_dram_parameter("ones", [P, P], F32R, isOutput=False)
    ident_ext = nc.declare_dram_parameter("ident", [P, P], BF16, isOutput=False)
    out_ext = nc.declare_dram_parameter("out", [H, T], F32, isOutput=True)

    xT_r = xT_ext.rearrange("(ct p) t -> p ct t", p=P)
    w_r = {n: w_ext[n].rearrange("(ct p) h -> p ct h", p=P) for n in w_ext}

    with TileContext(nc) as tc:
        with (
            tc.tile_pool(name="const", bufs=1) as const,
            tc.tile_pool(name="kt", bufs=NCH) as kt_pool,
            tc.tile_pool(name="vnat", bufs=16) as v_pool,
            tc.tile_pool(name="xch", bufs=2) as x_pool,
            tc.tile_pool(name="qv", bufs=2) as qv_pool,
            tc.tile_pool(name="ex", bufs=8) as e_pool,
            tc.tile_pool(name="dacc", bufs=2) as acc_pool,
            tc.tile_pool(name="outp", bufs=2) as out_pool,
            tc.tile_pool(name="ps_proj", bufs=1, space="PSUM") as proj_ps,
            tc.tile_pool(name="ps_sc", bufs=3, space="PSUM") as sc_ps,
            tc.tile_pool(name="ps_o", bufs=2, space="PSUM") as o_ps,
            tc.tile_pool(name="ps_d", bufs=1, space="PSUM") as d_ps,
            tc.tile_pool(name="ps_tr", bufs=1, space="PSUM") as tr_ps,
        ):
            # ---- constants; ones/ident first (warmup), then Wq + x0
            w_sb = {}
            for n in ("Wq", "Wk", "Wv"):
                w_sb[n] = [
                    const.tile([P, H], F16, tag=f"w_{n}_{c}", name=f"w_{n}_{c}")
                    for c in range(CT)
                ]
            b_sb = {
                n: const.tile([H, 1], F32, tag=f"b_{n}", name=f"b_{n}")
                for n in ("bq", "bv")
            }
            ones_r = const.tile([P, P], F32R, tag="ones_r")
            nc.sync.dma_start(ones_r[:], ones_ext[:])
            ident = const.tile([P, P], BF16, tag="ident")
            nc.sync.dma_start(ident[:], ident_ext[:])
            x_tiles = [None] * NCH

            def emit_x_dma(j):
                tiles = []
                tsl = slice(CHW * j, CHW * (j + 1))
                for c in range(CT):
                    xt = x_pool.tile([P, CHW], F16, tag=f"xc{c}", name=f"x{j}_{c}")
                    nc.sync.dma_start(xt[:], xT_r[:, c, tsl])
                    tiles.append(xt)
                x_tiles[j] = tiles

            for c in range(CT):
                nc.sync.dma_start(w_sb["Wq"][c][:], w_r["Wq"][:, c, :])
            emit_x_dma(0)
            nc.sync.dma_start(b_sb["bq"][:], b_ext["bq"][:])
            nc.sync.dma_start(b_sb["bv"][:], b_ext["bv"][:])
            tril = const.tile([P, P], BF16, tag="tril")
            nc.sync.dma_start(tril[:], tril_ext[:])
            for c in range(CT):
                nc.sync.dma_start(w_sb["Wk"][c][:], w_r["Wk"][:, c, :])
            for c in range(CT):
                nc.sync.dma_start(w_sb["Wv"][c][:], w_r["Wv"][:, c, :])
            emit_x_dma(1)

            # PE warmup over the DMA prologue; bf16 memset tiles so the PE
            # can start before any DMA lands.
            ones_bf = const.tile([P, P], BF16, tag="ones_bf")
            nc.vector.memset(ones_bf[:], 1.0)
            warm_src = const.tile([P, CHW], BF16, tag="warm_src")
            nc.vector.memset(warm_src[:], 0.0)
            ps_warm = sc_ps.tile([P, CHW], F32, tag="sc", name="ps_warm")

            def warm(n=1, width=CHW):
                for _ in range(n):
                    nc.tensor.matmul(
                        ps_warm[:, :width], ones_bf[:], warm_src[:, :width],
                        start=True, stop=True,
                    )

            warm(WARMUP)

            kt_ch = [None] * NCH
            v_tiles = [None] * (NCH * SPC)
            q_chs = [None] * NCH
            state = {}

            def proj_units(j):
                """Unit thunks: q chain(+tail), k chain(+tail), v chain(+tail),
                then the 4 v transposes."""
                units = []
                ps_tiles = {}
                xt = x_tiles[j]

                def mk_mm(kind, wname, c):
                    def f():
                        if c == 0:
                            ps_tiles[kind] = proj_ps.tile(
                                [P, CHW], F32, tag="proj", name=f"ps_{kind}{j}"
                            )
                        nc.tensor.matmul(
                            ps_tiles[kind][:],
                            w_sb[wname][c][:],
                            xt[c][:],
                            start=(c == 0),
                            stop=(c == CT - 1),
                        )
                    return f

                def q_tail():
                    q = qv_pool.tile([P, CHW], F32R, tag="qch", name=f"q{j}")
                    nc.vector.tensor_scalar_add(q[:], ps_tiles["q"][:], b_sb["bq"][:])
                    q_chs[j] = q

                def k_tail():
                    kt = kt_pool.tile([P, CHW], F32R, tag="ktch", name=f"kt{j}")
                    nc.vector.tensor_copy(kt[:], ps_tiles["k"][:])
                    kt_ch[j] = kt

                def v_tail():
                    vch = qv_pool.tile([P, CHW], BF16, tag="vch", name=f"v{j}")
                    nc.vector.tensor_scalar_add(
                        vch[:], ps_tiles["v"][:], b_sb["bv"][:]
                    )
                    ps_tiles["vch"] = vch

                def mk_vt(st):
                    def f():
                        ps_t = tr_ps.tile([P, P], BF16, tag="tr")
                        nc.tensor.transpose(
                            ps_t[:],
                            ps_tiles["vch"][:, P * st : P * (st + 1)],
                            ident[:],
                        )
                        vt = v_pool.tile(
                            [P, P], BF16, tag="vnat", name=f"vnat_{SPC*j+st}"
                        )
                        nc.vector.tensor_copy(vt[:], ps_t[:])
                        v_tiles[SPC * j + st] = vt
                    return f

                for kind, wname, tail in (
                    ("q", "Wq", q_tail),
                    ("k", "Wk", k_tail),
                    ("v", "Wv", v_tail),
                ):
                    for c in range(CT):
                        units.append(mk_mm(kind, wname, c))
                    units.append(tail)
                for st in range(SPC):
                    units.append(mk_vt(st))
                return units

            def make_state(j):
                n_s = SPC * (j + 1)
                n_off = SPC * j
                s = {"n_s": n_s, "n_off": n_off, "pending": []}
                if n_off:
                    s["acc"] = acc_pool.tile(
                        [P, CHW], BF16, tag="dacc", name=f"acc{j}"
                    )
                s["ps_o"] = o_ps.tile([P, CHW], F32, tag="o", name=f"o{j}")
                s["ps_d"] = d_ps.tile([P, CHW], F32, tag="d", name=f"d{j}")
                state[j] = s

            def emit_step(j, i):
                s = state[j]
                n_s, n_off = s["n_s"], s["n_off"]
                diag = i >= n_off
                st = i - n_off
                w0 = P * st if diag else 0
                w0sc = w0

                ps_sc = sc_ps.tile([P, CHW], F32, tag="sc", name=f"sc{j}_{i}")
                nc.tensor.matmul(
                    ps_sc[:, w0sc:],
                    kt_ch[i // SPC][:, P * (i % SPC) : P * (i % SPC + 1)],
                    q_chs[j][:, w0sc:],
                    start=True,
                    stop=True,
                )
                eb = e_pool.tile([P, CHW], BF16, tag="e", name=f"e{j}_{i}")
                nc.scalar.activation(eb[:, w0:], ps_sc[:, w0:], Exp)
                if diag:
                    nc.vector.tensor_tensor(
                        eb[:, w0 : w0 + P], eb[:, w0 : w0 + P], tril[:], Mult
                    )
                else:
                    if i == 0:
                        nc.vector.tensor_copy(s["acc"][:], eb[:])
                    else:
                        nc.vector.tensor_tensor(s["acc"][:], s["acc"][:], eb[:], Add)

                vt = v_tiles[i]

                def pend(eb=eb, vt=vt, w0=w0, i=i, diag=diag, st=st):
                    nc.tensor.matmul(
                        s["ps_o"][:, w0:],
                        vt[:],
                        eb[:, w0:],
                        start=(i == 0),
                        stop=(i == n_s - 1),
                    )
                    if diag:
                        if st == 0 and n_off:
                            nc.tensor.matmul(
                                s["ps_d"][:], ones_bf[:], s["acc"][:],
                                start=True, stop=False,
                            )
                        nc.tensor.matmul(
                            s["ps_d"][:, w0:], ones_bf[:], eb[:, w0:],
                            start=(st == 0 and not n_off),
                            stop=(st == SPC - 1),
                        )
                s["pending"].append(pend)
                if len(s["pending"]) > 3:
                    s["pending"].pop(0)()

            def drain(j, fills=None):
                pend = state[j]["pending"]
                state[j]["pending"] = []
                for n, f in enumerate(pend):
                    f()
                    if fills and n == 0:
                        for g in fills:
                            g()

            def emit_end(j, nsplit):
                tsl0 = CHW * j
                s = state[j]
                recip = out_pool.tile([P, CHW], F32, tag="recip", name=f"rc{j}")
                o_sb = out_pool.tile([P, CHW], F32, tag="osb", name=f"ob{j}")
                w = CHW // nsplit
                for sp in range(nsplit):
                    sl = slice(w * sp, w * (sp + 1))
                    nc.vector.reciprocal_approx_fast(
                        out=recip[:, sl], in_=s["ps_d"][:, sl]
                    )
                    nc.vector.tensor_tensor(
                        o_sb[:, sl], s["ps_o"][:, sl], recip[:, sl], Mult
                    )
                    nc.sync.dma_start(
                        out_ext[:, tsl0 + w * sp : tsl0 + w * (sp + 1)], o_sb[:, sl]
                    )

            # ---- prologue: chunk 0 projections; keep the PE clock up with
            # filler matmuls while the x0 c-tiles land
            make_state(0)
            p0 = proj_units(0)
            for n, u in enumerate(p0):
                if n <= CT:
                    warm(3)
                u()

            # ---- phases: chunk j diagonal + chunk j+1 off-diagonal + proj j+1
            for j in range(NCH):
                has_next = j + 1 < NCH
                if j + 2 < NCH:
                    emit_x_dma(j + 2)
                D = list(range(SPC * j, SPC * (j + 1)))
                if has_next:
                    make_state(j + 1)
                    F = proj_units(j + 1)
                    O = list(range(SPC * (j + 1)))
                else:
                    F, O = [], []

                # part 1: chunk j diagonal steps woven with the start of the
                # next chunk's q projection chain
                q_chain, rest = F[: CT + 1], F[CT + 1 :]
                qi = 0
                for d_idx, i in enumerate(D):
                    emit_step(j, i)
                    if j == 0:
                        continue  # x1 still landing; don't block the PE queue
                    take = (len(q_chain) * (d_idx + 1)) // max(1, len(D)) - qi
                    for _ in range(take):
                        q_chain[qi]()
                        qi += 1
                while qi < len(q_chain):
                    q_chain[qi]()
                    qi += 1
                drain(j, fills=rest[:2])
                rest = rest[2:]
                emit_end(j, 2 if has_next else 4)

                # part 2: chunk j+1 off-diagonal steps woven with its k/v
                # projection chains and v transposes
                # front-load: consume all rest units by ~70% through O
                ri = 0
                n_o = max(1, (len(O) * 7) // 10)
                for o_idx, i in enumerate(O):
                    emit_step(j + 1, i)
                    if j == 0:
                        warm(1)
                    take = (len(rest) * min(o_idx + 1, n_o)) // n_o - ri
                    for _ in range(take):
                        rest[ri]()
                        ri += 1
                while ri < len(rest):
                    rest[ri]()
                    ri += 1

    nc.compile()
    _BUILT = nc
    return nc


def _host_inputs(x, Wq, bq, Wk, bk, Wv, bv):
    import ml_dtypes

    tril = (np.arange(P)[:, None] <= np.arange(P)[None, :]).astype(
        ml_dtypes.bfloat16
    )
    shared = {
        "Wq": np.ascontiguousarray(Wq, dtype=np.float16),
        "Wk": np.ascontiguousarray(Wk, dtype=np.float16),
        "Wv": np.ascontiguousarray(Wv, dtype=np.float16),
        "bq": np.ascontiguousarray(bq, dtype=np.float32).reshape(H, 1),
        "bv": np.ascontiguousarray(bv, dtype=np.float32).reshape(H, 1),
        "tril": tril,
        "ones": np.ones((P, P), dtype=np.float32),
        "ident": np.eye(P, dtype=np.float32).astype(ml_dtypes.bfloat16),
    }
    in_maps = []
    for b in range(B):
        m = dict(shared)
        m["xT"] = np.ascontiguousarray(np.asarray(x[b]).T.astype(np.float16))
        in_maps.append(m)
    return in_maps


def kernel(x, Wq, bq, Wk, bk, Wv, bv):
    global LAST_EXEC_TIME_NS
    from concourse.bass_utils import run_bass_kernel_spmd

    nc = _build()
    in_maps = _host_inputs(x, Wq, bq, Wk, bk, Wv, bv)
    trace = os.environ.get("BASS_ATTN_TRACE", "0") == "1"
    res = run_bass_kernel_spmd(nc, in_maps, core_ids=list(range(N_CORES)), trace=trace)
    LAST_EXEC_TIME_NS = res.exec_time_ns
    out = np.stack([res.results[b]["out"].T for b in range(B)], axis=0)
    return np.ascontiguousarray(out, dtype=np.float32)


# revision 39
# speedup vs baseline: 1.0763x; 1.0110x over previous
"""Single-head causal attention (B=8, T=2048, C=1024, H=128) on 8 TRN2 NeuronCores.

Sharding: data-parallel over batch — core b computes batch element b entirely
(no collectives). Host pre-transposes x[b] to xT=[C,T]; the device returns
out^T=[H,T] which the host transposes back.

Schedule: all-f32r PE stream, software-pipelined across chunks. Phase j
emits chunk j's diagonal attention steps, chunk j+1's off-diagonal steps,
and chunk j+1's projection matmuls, woven so the in-order PE queue always
has ready work while ACT exp latency drains. AV and denominator (ones)
matmuls trail their score step by 3 (sc_ps bufs=3), accumulating into
per-chunk ps_o / ps_d banks with causal width-trimming. bk is dropped
(softmax shift invariance); bias adds and PSUM->SBUF copies run on DVE so
ACT is a pure exp stream. Warmup matmuls are woven through the DMA-paced
prologue to hold the PE clock up.
"""

import os
import numpy as np

T, C, H = 2048, 1024, 128
B = 8
P = 128
CT = C // P          # 8 contraction tiles
NCH = 4              # t-chunks
CHW = T // NCH       # 512 chunk width
SPC = CHW // P       # 4 s-tiles per chunk
N_CORES = 8
WARMUP = 10

LAST_EXEC_TIME_NS = None

_BUILT = None


def _build():
    global _BUILT
    if _BUILT is not None:
        return _BUILT

    import concourse.bass as bass  # noqa: F401
    import concourse.mybir as mybir
    from concourse import bacc
    from concourse.tile import TileContext

    F32 = mybir.dt.float32
    F16 = mybir.dt.float16
    F32R = mybir.dt.float32r
    BF16 = mybir.dt.bfloat16
    Identity = mybir.ActivationFunctionType.Identity
    Exp = mybir.ActivationFunctionType.Exp
    Mult = mybir.AluOpType.mult
    Add = mybir.AluOpType.add

    nc = bacc.Bacc()

    xT_ext = nc.declare_dram_parameter("xT", [C, T], F16, isOutput=False)
    w_ext = {
        n: nc.declare_dram_parameter(n, [C, H], F16, isOutput=False)
        for n in ("Wq", "Wk", "Wv")
    }
    b_ext = {
        n: nc.declare_dram_parameter(n, [H, 1], F32, isOutput=False)
        for n in ("bq", "bv")
    }
    tril_ext = nc.declare_dram_parameter("tril", [P, P], BF16, isOutput=False)
    ones_ext = nc.declare_dram_parameter("ones", [P, P], F32R, isOutput=False)
    ident_ext = nc.declare_dram_parameter("ident", [P, P], BF16, isOutput=False)
    out_ext = nc.declare_dram_parameter("out", [H, T], F32, isOutput=True)

    xT_r = xT_ext.rearrange("(ct p) t -> p ct t", p=P)
    w_r = {n: w_ext[n].rearrange("(ct p) h -> p ct h", p=P) for n in w_ext}

    with TileContext(nc) as tc:
        with (
            tc.tile_pool(name="const", bufs=1) as const,
            tc.tile_pool(name="kt", bufs=NCH) as kt_pool,
            tc.tile_pool(name="vnat", bufs=16) as v_pool,
            tc.tile_pool(name="xch", bufs=2) as x_pool,
            tc.tile_pool(name="qv", bufs=2) as qv_pool,
            tc.tile_pool(name="ex", bufs=8) as e_pool,
            tc.tile_pool(name="dacc", bufs=2) as acc_pool,
            tc.tile_pool(name="outp", bufs=2) as out_pool,
            tc.tile_pool(name="ps_proj", bufs=1, space="PSUM") as proj_ps,
            tc.tile_pool(name="ps_sc", bufs=3, space="PSUM") as sc_ps,
            tc.tile_pool(name="ps_o", bufs=2, space="PSUM") as o_ps,
            tc.tile_pool(name="ps_d", bufs=1, space="PSUM") as d_ps,
            tc.tile_pool(name="ps_tr", bufs=1, space="PSUM") as tr_ps,
        ):
            # ---- constants; ones/ident first (warmup), then Wq + x0
            w_sb = {}
            for n in ("Wq", "Wk", "Wv"):
                w_sb[n] = [
                    const.tile([P, H], F16, tag=f"w_{n}_{c}", name=f"w_{n}_{c}")
                    for c in range(CT)
                ]
            b_sb = {
                n: const.tile([H, 1], F32, tag=f"b_{n}", name=f"b_{n}")
                for n in ("bq", "bv")
            }
            ones_r = const.tile([P, P], F32R, tag="ones_r")
            nc.sync.dma_start(ones_r[:], ones_ext[:])
            ident = const.tile([P, P], BF16, tag="ident")
            nc.sync.dma_start(ident[:], ident_ext[:])
            x_tiles = [None] * NCH

            def emit_x_dma(j):
                tiles = []
                tsl = slice(CHW * j, CHW * (j + 1))
                for c in range(CT):
                    xt = x_pool.tile([P, CHW], F16, tag=f"xc{c}", name=f"x{j}_{c}")
                    nc.sync.dma_start(xt[:], xT_r[:, c, tsl])
                    tiles.append(xt)
                x_tiles[j] = tiles

            for c in range(CT):
                nc.sync.dma_start(w_sb["Wq"][c][:], w_r["Wq"][:, c, :])
            emit_x_dma(0)
            nc.sync.dma_start(b_sb["bq"][:], b_ext["bq"][:])
            nc.sync.dma_start(b_sb["bv"][:], b_ext["bv"][:])
            tril = const.tile([P, P], BF16, tag="tril")
            nc.sync.dma_start(tril[:], tril_ext[:])
            for c in range(CT):
                nc.sync.dma_start(w_sb["Wk"][c][:], w_r["Wk"][:, c, :])
            for c in range(CT):
                nc.sync.dma_start(w_sb["Wv"][c][:], w_r["Wv"][:, c, :])
            emit_x_dma(1)

            # PE warmup over the DMA prologue; bf16 memset tiles so the PE
            # can start before any DMA lands.
            ones_bf = const.tile([P, P], BF16, tag="ones_bf")
            nc.vector.memset(ones_bf[:], 1.0)
            warm_src = const.tile([P, CHW], BF16, tag="warm_src")
            nc.vector.memset(warm_src[:], 0.0)
            ps_warm = sc_ps.tile([P, CHW], F32, tag="sc", name="ps_warm")

            def warm(n=1, width=CHW):
                for _ in range(n):
                    nc.tensor.matmul(
                        ps_warm[:, :width], ones_bf[:], warm_src[:, :width],
                        start=True, stop=True,
                    )

            warm(WARMUP)

            kt_ch = [None] * NCH
            v_tiles = [None] * (NCH * SPC)
            q_chs = [None] * NCH
            state = {}

            def proj_units(j):
                """Unit thunks: q chain(+tail), k chain(+tail), v chain(+tail),
                then the 4 v transposes."""
                units = []
                ps_tiles = {}
                xt = x_tiles[j]

                def mk_mm(kind, wname, c):
                    def f():
                        if c == 0:
                            ps_tiles[kind] = proj_ps.tile(
                                [P, CHW], F32, tag="proj", name=f"ps_{kind}{j}"
                            )
                        nc.tensor.matmul(
                            ps_tiles[kind][:],
                            w_sb[wname][c][:],
                            xt[c][:],
                            start=(c == 0),
                            stop=(c == CT - 1),
                        )
                    return f

                def q_tail():
                    q = qv_pool.tile([P, CHW], F32R, tag="qch", name=f"q{j}")
                    nc.vector.tensor_scalar_add(q[:], ps_tiles["q"][:], b_sb["bq"][:])
                    q_chs[j] = q

                def k_tail():
                    kt = kt_pool.tile([P, CHW], F32R, tag="ktch", name=f"kt{j}")
                    nc.vector.tensor_copy(kt[:], ps_tiles["k"][:])
                    kt_ch[j] = kt

                def v_tail():
                    vch = qv_pool.tile([P, CHW], BF16, tag="vch", name=f"v{j}")
                    nc.vector.tensor_scalar_add(
                        vch[:], ps_tiles["v"][:], b_sb["bv"][:]
                    )
                    ps_tiles["vch"] = vch

                def mk_vt(st):
                    def f():
                        ps_t = tr_ps.tile([P, P], BF16, tag="tr")
                        nc.tensor.transpose(
                            ps_t[:],
                            ps_tiles["vch"][:, P * st : P * (st + 1)],
                            ident[:],
                        )
                        vt = v_pool.tile(
                            [P, P], BF16, tag="vnat", name=f"vnat_{SPC*j+st}"
                        )
                        nc.vector.tensor_copy(vt[:], ps_t[:])
                        v_tiles[SPC * j + st] = vt
                    return f

                for kind, wname, tail in (
                    ("q", "Wq", q_tail),
                    ("k", "Wk", k_tail),
                    ("v", "Wv", v_tail),
                ):
                    for c in range(CT):
                        units.append(mk_mm(kind, wname, c))
                    units.append(tail)
                for st in range(SPC):
                    units.append(mk_vt(st))
                return units

            def make_state(j):
                n_s = SPC * (j + 1)
                n_off = SPC * j
                s = {"n_s": n_s, "n_off": n_off, "pending": []}
                if n_off:
                    s["acc"] = acc_pool.tile(
                        [P, CHW], BF16, tag="dacc", name=f"acc{j}"
                    )
                s["ps_o"] = o_ps.tile([P, CHW], F32, tag="o", name=f"o{j}")
                s["ps_d"] = d_ps.tile([P, CHW], F32, tag="d", name=f"d{j}")
                state[j] = s

            def emit_step(j, i):
                s = state[j]
                n_s, n_off = s["n_s"], s["n_off"]
                diag = i >= n_off
                st = i - n_off
                w0 = P * st if diag else 0
                w0sc = w0

                ps_sc = sc_ps.tile([P, CHW], F32, tag="sc", name=f"sc{j}_{i}")
                nc.tensor.matmul(
                    ps_sc[:, w0sc:],
                    kt_ch[i // SPC][:, P * (i % SPC) : P * (i % SPC + 1)],
                    q_chs[j][:, w0sc:],
                    start=True,
                    stop=True,
                )
                eb = e_pool.tile([P, CHW], BF16, tag="e", name=f"e{j}_{i}")
                nc.scalar.activation(eb[:, w0:], ps_sc[:, w0:], Exp)
                if diag:
                    nc.vector.tensor_tensor(
                        eb[:, w0 : w0 + P], eb[:, w0 : w0 + P], tril[:], Mult
                    )
                else:
                    if i == 0:
                        nc.vector.tensor_copy(s["acc"][:], eb[:])
                    else:
                        nc.vector.tensor_tensor(s["acc"][:], s["acc"][:], eb[:], Add)

                vt = v_tiles[i]

                def pend(eb=eb, vt=vt, w0=w0, i=i, diag=diag, st=st):
                    nc.tensor.matmul(
                        s["ps_o"][:, w0:],
                        vt[:],
                        eb[:, w0:],
                        start=(i == 0),
                        stop=(i == n_s - 1),
                    )
                    if diag:
                        if st == 0 and n_off:
                            nc.tensor.matmul(
                                s["ps_d"][:], ones_bf[:], s["acc"][:],
                                start=True, stop=False,
                            )
                        nc.tensor.matmul(
                            s["ps_d"][:, w0:], ones_bf[:], eb[:, w0:],
                            start=(st == 0 and not n_off),
                            stop=(st == SPC - 1),
                        )
                s["pending"].append(pend)
                if len(s["pending"]) > 3:
                    s["pending"].pop(0)()

            def drain(j, fills=None):
                pend = state[j]["pending"]
                state[j]["pending"] = []
                for n, f in enumerate(pend):
                    f()
                    if fills and n == 0:
                        for g in fills:
                            g()

            def emit_end(j, nsplit):
                tsl0 = CHW * j
                s = state[j]
                recip = out_pool.tile([P, CHW], F32, tag="recip", name=f"rc{j}")
                o_sb = out_pool.tile([P, CHW], F32, tag="osb", name=f"ob{j}")
                w = CHW // nsplit
                for sp in range(nsplit):
                    sl = slice(w * sp, w * (sp + 1))
                    nc.vector.reciprocal_approx_fast(
                        out=recip[:, sl], in_=s["ps_d"][:, sl]
                    )
                    nc.vector.tensor_tensor(
                        o_sb[:, sl], s["ps_o"][:, sl], recip[:, sl], Mult
                    )
                    nc.sync.dma_start(
                        out_ext[:, tsl0 + w * sp : tsl0 + w * (sp + 1)], o_sb[:, sl]
                    )

            # ---- prologue: chunk 0 projections; keep the PE clock up with
            # filler matmuls while the x0 c-tiles land
            make_state(0)
            p0 = proj_units(0)
            for n, u in enumerate(p0):
                if n <= CT:
                    warm(3 if n < 4 else 5)
                u()

            # ---- phases: chunk j diagonal + chunk j+1 off-diagonal + proj j+1
            for j in range(NCH):
                has_next = j + 1 < NCH
                if j + 2 < NCH:
                    emit_x_dma(j + 2)
                D = list(range(SPC * j, SPC * (j + 1)))
                if has_next:
                    make_state(j + 1)
                    F = proj_units(j + 1)
                    O = list(range(SPC * (j + 1)))
                else:
                    F, O = [], []

                # part 1: chunk j diagonal steps woven with the start of the
                # next chunk's q projection chain
                q_chain, rest = F[: CT + 1], F[CT + 1 :]
                qi = 0
                for d_idx, i in enumerate(D):
                    emit_step(j, i)
                    if j == 0:
                        continue  # x1 still landing; don't block the PE queue
                    take = (len(q_chain) * (d_idx + 1)) // max(1, len(D)) - qi
                    for _ in range(take):
                        q_chain[qi]()
                        qi += 1
                while qi < len(q_chain):
                    q_chain[qi]()
                    qi += 1
                drain(j, fills=rest[:2])
                rest = rest[2:]
                emit_end(j, 2 if has_next else 4)

                # part 2: chunk j+1 off-diagonal steps woven with its k/v
                # projection chains and v transposes
                # front-load: consume all rest units by ~70% through O
                ri = 0
                n_o = max(1, (len(O) * 7) // 10)
                for o_idx, i in enumerate(O):
                    emit_step(j + 1, i)
                    if j == 0:
                        warm(2)
                    take = (len(rest) * min(o_idx + 1, n_o)) // n_o - ri
                    for _ in range(take):
                        rest[ri]()
                        ri += 1
                while ri < len(rest):
                    rest[ri]()
                    ri += 1

    nc.compile()
    _BUILT = nc
    return nc


def _host_inputs(x, Wq, bq, Wk, bk, Wv, bv):
    import ml_dtypes

    tril = (np.arange(P)[:, None] <= np.arange(P)[None, :]).astype(
        ml_dtypes.bfloat16
    )
    shared = {
        "Wq": np.ascontiguousarray(Wq, dtype=np.float16),
        "Wk": np.ascontiguousarray(Wk, dtype=np.float16),
        "Wv": np.ascontiguousarray(Wv, dtype=np.float16),
        "bq": np.ascontiguousarray(bq, dtype=np.float32).reshape(H, 1),
        "bv": np.ascontiguousarray(bv, dtype=np.float32).reshape(H, 1),
        "tril": tril,
        "ones": np.ones((P, P), dtype=np.float32),
        "ident": np.eye(P, dtype=np.float32).astype(ml_dtypes.bfloat16),
    }
    in_maps = []
    for b in range(B):
        m = dict(shared)
        m["xT"] = np.ascontiguousarray(np.asarray(x[b]).T.astype(np.float16))
        in_maps.append(m)
    return in_maps


def kernel(x, Wq, bq, Wk, bk, Wv, bv):
    global LAST_EXEC_TIME_NS
    from concourse.bass_utils import run_bass_kernel_spmd

    nc = _build()
    in_maps = _host_inputs(x, Wq, bq, Wk, bk, Wv, bv)
    trace = os.environ.get("BASS_ATTN_TRACE", "0") == "1"
    res = run_bass_kernel_spmd(nc, in_maps, core_ids=list(range(N_CORES)), trace=trace)
    LAST_EXEC_TIME_NS = res.exec_time_ns
    out = np.stack([res.results[b]["out"].T for b in range(B)], axis=0)
    return np.ascontiguousarray(out, dtype=np.float32)
